# revision 1
# baseline (speedup 1.0000x reference)
"""Trainium2 Bass kernel for 16-head MHA (B=2, S=4096, D=1024).

Sharding: 8 cores = 2 batches x 4 head-groups (4 heads each).
Each core computes, for its (batch b, head group g):
    Q^T/K^T ([256, S] in head-major layout), V ([S, 256] + ones cols),
    per head: S^T = K Q^T (scores transposed), P = exp(S^T/8),
    [O^T; D] = [V|1]^T @ P^T  (PV matmul with fused denominator row),
    O^T_norm = O^T / D, Y^T_partial = woT^T @ O^T_norm.
Host sums the 4 per-head-group partials per batch and adds b_o.

All inputs arrive host-side pre-transposed so every DMA is contiguous.
Activations are stored in fine-grained tiles (per 512-col chunk / per
128-row chunk) so the Tile scheduler overlaps the projection, attention,
and output-projection phases instead of serializing them.
"""

import os
import sys

sys.path.insert(0, "/opt/trn_rl_repo")
os.environ.setdefault("MYCRO_LOCAL_CACHE", "1")

from contextlib import ExitStack

import numpy as np

import concourse.bass as bass
import concourse.tile as tile
from concourse import bacc, mybir

F32 = mybir.dt.float32
BF16 = mybir.dt.bfloat16
AF = mybir.ActivationFunctionType
ALU = mybir.AluOpType

D = 1024  # d_model
NH = 16  # total heads
DH = 64  # head dim
HPC = 4  # heads per core
MG = HPC * DH  # 256 model cols per core


def build_module(S: int = 4096) -> bass.Bass:
    nc = bacc.Bacc("TRN2", target_bir_lowering=False, debug=False, num_devices=8)

    xq = nc.dram_tensor("xqt", [D, S], F32, kind="ExternalInput")  # q[b].T
    xk = nc.dram_tensor("xkt", [D, S], F32, kind="ExternalInput")
    xv = nc.dram_tensor("xvt", [D, S], F32, kind="ExternalInput")
    wq = nc.dram_tensor("wqt", [D, MG], F32, kind="ExternalInput")  # w_q[rows_g].T
    wk = nc.dram_tensor("wkt", [D, MG], F32, kind="ExternalInput")
    wv = nc.dram_tensor("wvt", [D, MG], F32, kind="ExternalInput")
    wo = nc.dram_tensor("wot", [MG, D], F32, kind="ExternalInput")  # w_o[:, cols_g].T
    bq = nc.dram_tensor("bq", [MG], F32, kind="ExternalInput")
    bk = nc.dram_tensor("bk", [MG], F32, kind="ExternalInput")
    bv = nc.dram_tensor("bv", [MG], F32, kind="ExternalInput")
    yt = nc.dram_tensor("yt", [D, S], F32, kind="ExternalOutput")  # partial y[b].T

    SC = min(1024, S)  # attention s-chunk width
    n_sc = S // SC
    n_tc = S // 128  # key/value chunks of 128
    PSC = min(512, S)  # projection s-chunk
    n_psc = S // PSC
    VTG = min(1024, S)  # v-projection t group width
    n_vtg = S // VTG
    ND = D // 128  # d-model tiles

    with tile.TileContext(nc) as tc, ExitStack() as ctx:
        persist = ctx.enter_context(tc.tile_pool(name="persist", bufs=1))

        # -------- weights / biases to SBUF --------
        wq_s = persist.tile([128, ND, MG], BF16, tag="wq")
        wk_s = persist.tile([128, ND, MG], BF16, tag="wk")
        wv_s = persist.tile([128, ND, MG], BF16, tag="wv")
        wo_s = persist.tile([128, MG // 128, D], BF16, tag="wo")
        bq_r = persist.tile([1, MG], BF16, tag="bq")
        bk_r = persist.tile([1, MG], BF16, tag="bk")
        bv_r = persist.tile([1, MG], BF16, tag="bv")
        ones_r = persist.tile([1, PSC], BF16, tag="ones_r")
        nc.gpsimd.dma_start(wq_s[:], wq[:].rearrange("(d p) m -> p d m", p=128))
        nc.gpsimd.dma_start(wk_s[:], wk[:].rearrange("(d p) m -> p d m", p=128))
        nc.gpsimd.dma_start(wv_s[:], wv[:].rearrange("(d p) m -> p d m", p=128))
        nc.gpsimd.dma_start(wo_s[:], wo[:].rearrange("(t p) n -> p t n", p=128))
        nc.gpsimd.dma_start(bq_r[:], bq[:].unsqueeze(0))
        nc.gpsimd.dma_start(bk_r[:], bk[:].unsqueeze(0))
        nc.gpsimd.dma_start(bv_r[:], bv[:].unsqueeze(0))
        nc.vector.memset(ones_r[:], 1.0)

        # -------- persistent activations (fine-grained tiles) --------
        # Q^T/K^T: [hp][psc] tiles of [128, PSC] (partitions = 2 heads x 64)
        qts = [
            [persist.tile([128, PSC], BF16, tag=f"qt{i}_{j}", name=f"qt{i}_{j}")
             for j in range(n_psc)]
            for i in range(2)
        ]
        kts = [
            [persist.tile([128, PSC], BF16, tag=f"kt{i}_{j}", name=f"kt{i}_{j}")
             for j in range(n_psc)]
            for i in range(2)
        ]
        # V: per-tc tiles [t=128, 4*(64+1)]; col 64 of each head's group = ones
        vst = [
            persist.tile([128, HPC * (DH + 1)], BF16, tag=f"vs{j}", name=f"vs{j}")
            for j in range(n_tc)
        ]
        # O^T: per (hp, sc) tiles [128, SC]
        ott = [
            [persist.tile([128, SC], BF16, tag=f"ot{i}_{j}", name=f"ot{i}_{j}")
             for j in range(n_sc)]
            for i in range(2)
        ]

        for j in range(n_tc):
            for h in range(HPC):
                nc.vector.memset(vst[j][:, h * 65 + 64 : h * 65 + 65], 1.0)

        with tc.tile_pool(name="v_stage", bufs=10) as vstage, tc.tile_pool(
            name="qk_stage", bufs=10
        ) as stage, tc.tile_pool(
            name="qk_psum", bufs=2, space="PSUM"
        ) as qkp, tc.tile_pool(
            name="pv_psum", bufs=2, space="PSUM"
        ) as pvp, tc.tile_pool(name="pt_pool", bufs=4) as ptp, tc.tile_pool(
            name="norm", bufs=4
        ) as normp, tc.tile_pool(name="y_stage", bufs=4) as ysp:
            # -------- phase A: V projection (natural [t, m] layout) --------
            for tg in range(n_vtg):
                xv_t = []
                for d in range(ND):
                    t1 = vstage.tile([128, VTG], BF16, tag="xv", name="xv")
                    nc.gpsimd.dma_start(
                        t1[:], xv[d * 128 : (d + 1) * 128, tg * VTG : (tg + 1) * VTG]
                    )
                    xv_t.append(t1)
                for tl in range(VTG // 128):
                    ps = qkp.tile([128, MG], F32, tag="qk", name="pjv")
                    for d in range(ND):
                        nc.tensor.matmul(
                            ps[:],
                            xv_t[d][:, tl * 128 : (tl + 1) * 128],
                            wv_s[:, d, :],
                            start=(d == 0),
                            stop=False,
                        )
                    nc.tensor.matmul(
                        ps[:], ones_r[0:1, 0:128], bv_r[0:1, :], start=False, stop=True
                    )
                    tcix = tg * (VTG // 128) + tl
                    for h in range(HPC):
                        nc.vector.tensor_copy(
                            vst[tcix][:, h * 65 : h * 65 + 64],
                            ps[:, h * DH : (h + 1) * DH],
                        )

            # -------- phase B: K then Q projections --------
            for which, xin, w_s, b_r, dest in (
                ("k", xk, wk_s, bk_r, kts),
                ("q", xq, wq_s, bq_r, qts),
            ):
                for si in range(n_psc):
                    x_t = []
                    for d in range(ND):
                        t1 = stage.tile([128, PSC], BF16, tag=f"x{which}", name="xt")
                        nc.gpsimd.dma_start(
                            t1[:],
                            xin[d * 128 : (d + 1) * 128, si * PSC : (si + 1) * PSC],
                        )
                        x_t.append(t1)
                    for mc in range(MG // 128):
                        ps = qkp.tile([128, PSC], F32, tag="qk", name="pjq")
                        for d in range(ND):
                            nc.tensor.matmul(
                                ps[:],
                                w_s[:, d, mc * 128 : (mc + 1) * 128],
                                x_t[d][:],
                                start=(d == 0),
                                stop=False,
                            )
                        nc.tensor.matmul(
                            ps[:],
                            b_r[0:1, mc * 128 : (mc + 1) * 128],
                            ones_r[0:1, :],
                            start=False,
                            stop=True,
                        )
                        nc.vector.tensor_copy(dest[mc][si][:], ps[:])

            # -------- phase C: attention (si outer, hp inner) + out-proj --------
            if True:
                for si in range(n_sc):
                    for hp in range(2):
                        pv = [
                            pvp.tile([DH + 1, SC], F32, tag="pv", name="pv")
                            for _ in range(2)
                        ]
                        for tcix in range(n_tc):
                            kt_tile = kts[hp][(tcix * 128) // PSC]
                            kcol = (tcix * 128) % PSC
                            for hh in range(2):
                                po = DH * hh
                                qk = qkp.tile([128, SC], F32, tag="qk")
                                for nn in range(SC // 512):
                                    qt_tile = qts[hp][(si * SC + nn * 512) // PSC]
                                    qcol = (si * SC + nn * 512) % PSC
                                    nc.tensor.matmul(
                                        qk[:, nn * 512 : (nn + 1) * 512],
                                        kt_tile[po : po + DH, kcol : kcol + 128],
                                        qt_tile[po : po + DH, qcol : qcol + 512],
                                        start=True,
                                        stop=True,
                                    )
                                pt = ptp.tile([128, SC], BF16, tag="pt")
                                nc.scalar.activation(pt[:], qk[:], AF.Exp, scale=0.125)
                                h = hp * 2 + hh
                                for nn in range(SC // 512):
                                    nc.tensor.matmul(
                                        pv[hh][:, nn * 512 : (nn + 1) * 512],
                                        vst[tcix][:, h * 65 : (h + 1) * 65],
                                        pt[:, nn * 512 : (nn + 1) * 512],
                                        start=(tcix == 0),
                                        stop=(tcix == n_tc - 1),
                                    )
                        for hh in range(2):
                            po = DH * hh
                            dsb = normp.tile([1, SC], F32, tag="dsb", name="dsb")
                            nc.vector.tensor_copy(dsb[:], pv[hh][DH : DH + 1, :])
                            rd = normp.tile([1, SC], F32, tag="rd", name="rd")
                            nc.vector.reciprocal_approx_fast(rd[:], dsb[:])
                            rdb = normp.tile([DH, SC], F32, tag="rdb", name="rdb")
                            nc.gpsimd.partition_broadcast(rdb[:], rd[:])
                            nc.vector.tensor_tensor(
                                ott[hp][si][po : po + DH, :],
                                pv[hh][0:DH, :],
                                rdb[:],
                                ALU.mult,
                            )
                    # out-projection for this si (borrows the qk psum slots)
                    for nn8 in range(ND):
                        ps = qkp.tile([128, SC], F32, tag="qk", name="yp")
                        for mt in range(MG // 128):
                            for nn in range(SC // 512):
                                nc.tensor.matmul(
                                    ps[:, nn * 512 : (nn + 1) * 512],
                                    wo_s[:, mt, nn8 * 128 : (nn8 + 1) * 128],
                                    ott[mt][si][:, nn * 512 : (nn + 1) * 512],
                                    start=(mt == 0),
                                    stop=(mt == MG // 128 - 1),
                                )
                        ys = ysp.tile([128, SC], F32, tag="ys")
                        nc.vector.tensor_copy(ys[:], ps[:])
                        nc.sync.dma_start(
                            yt[nn8 * 128 : (nn8 + 1) * 128, si * SC : (si + 1) * SC],
                            ys[:],
                        )

    nc.compile()
    return nc


_MODULE_CACHE: dict = {}


def _get_module(S: int) -> bass.Bass:
    if S not in _MODULE_CACHE:
        _MODULE_CACHE[S] = build_module(S)
    return _MODULE_CACHE[S]


def make_in_maps(q, k, v, w_q, b_q, w_k, b_k, w_v, b_v, w_o, b_o):
    """Shard full inputs into 8 per-core input maps (host-side prep)."""
    f = lambda a: np.ascontiguousarray(np.asarray(a, dtype=np.float32))
    q, k, v = f(q), f(k), f(v)
    w_q, w_k, w_v, w_o = f(w_q), f(w_k), f(w_v), f(w_o)
    b_q, b_k, b_v = f(b_q), f(b_k), f(b_v)
    in_maps = []
    for core in range(8):
        b, g = core // 4, core % 4
        rows = slice(g * MG, (g + 1) * MG)
        in_maps.append(
            {
                "xqt": np.ascontiguousarray(q[b].T),
                "xkt": np.ascontiguousarray(k[b].T),
                "xvt": np.ascontiguousarray(v[b].T),
                "wqt": np.ascontiguousarray(w_q[rows].T),
                "wkt": np.ascontiguousarray(w_k[rows].T),
                "wvt": np.ascontiguousarray(w_v[rows].T),
                "wot": np.ascontiguousarray(w_o[:, rows].T),
                "bq": np.ascontiguousarray(b_q[rows]),
                "bk": np.ascontiguousarray(b_k[rows]),
                "bv": np.ascontiguousarray(b_v[rows]),
            }
        )
    return in_maps


def gather_output(results, b_o, B, S):
    y = np.zeros((B, S, D), np.float32)
    for core in range(8):
        b = core // 4
        y[b] += results[core]["yt"].T
    y += np.asarray(b_o, np.float32)[None, None, :]
    return y


def run(inputs: dict, trace: bool = False):
    """Run on 8 NeuronCores; returns (y, BassKernelResults)."""
    from concourse import bass_utils

    B, S, _ = np.asarray(inputs["q"]).shape
    mod = _get_module(S)
    in_maps = make_in_maps(**inputs)
    res = bass_utils.run_bass_kernel_spmd(
        mod, in_maps, core_ids=list(range(8)), trace=trace
    )
    y = gather_output(res.results, inputs["b_o"], B, S)
    return y, res


def kernel(q, k, v, w_q, b_q, w_k, b_k, w_v, b_v, w_o, b_o):
    y, _ = run(
        dict(
            q=q, k=k, v=v, w_q=w_q, b_q=b_q, w_k=w_k, b_k=b_k,
            w_v=w_v, b_v=b_v, w_o=w_o, b_o=b_o,
        )
    )
    return y



# revision 7
# speedup vs baseline: 1.3564x; 1.3564x over previous
"""Trainium2 Bass kernel for 16-head MHA (B=2, S=4096, D=1024).

Sharding: 8 cores = 2 batches x 4 head-groups (4 heads each).
Each core computes, for its (batch b, head group g):
    Q^T/K^T ([256, S] head-major), V ([S, 4*(64+1)] with ones cols),
    per head pair: S^T = K Q^T for both heads via concurrent row-tiled
    (K=64) matmuls into one [128, 1024] PSUM tile, one fused Exp over
    both heads, [O^T; D] = [V|1]^T P^T per head, O^T/D, Y^T = woT^T O^T.
Host sums the 4 per-head-group partials per batch and adds b_o.

Perf notes (from NTFF profile analysis):
  - PE matmul cost = N streamed rows regardless of K/M; the two K=64
    score matmuls of a head pair run CONCURRENTLY when issued
    back-to-back with lhsT/rhs at base partitions 0 and 64 (row
    tiling).  This halves score cost vs. serial emission.
  - The Act engine (only engine with Exp) is the bottleneck:
    512 x [128,1024] Exp instructions ~= 570us.  Keep everything else
    (copies, biases) off it: biases are folded into DVE evacuation.
  - PV is emitted lagged one t-block so the PE FIFO never blocks
    behind an Act-dependent instruction; Act runs back-to-back.
  - x/y are bf16 host-side to halve HBM traffic.
"""

import os
import sys

sys.path.insert(0, "/opt/trn_rl_repo")
os.environ.setdefault("MYCRO_LOCAL_CACHE", "1")

from contextlib import ExitStack

import ml_dtypes
import numpy as np

import concourse.bass as bass
import concourse.tile as tile
from concourse import bacc, mybir

F32 = mybir.dt.float32
BF16 = mybir.dt.bfloat16
AF = mybir.ActivationFunctionType
ALU = mybir.AluOpType

D = 1024  # d_model
NH = 16  # total heads
DH = 64  # head dim
HPC = 4  # heads per core
MG = HPC * DH  # 256 model cols per core

BF16NP = ml_dtypes.bfloat16


def build_module(S: int = 4096) -> bass.Bass:
    nc = bacc.Bacc("TRN2", target_bir_lowering=False, debug=False, num_devices=8)

    xq = nc.dram_tensor("xqt", [D, S], BF16, kind="ExternalInput")  # q[b].T
    xk = nc.dram_tensor("xkt", [D, S], BF16, kind="ExternalInput")
    xv = nc.dram_tensor("xvt", [D, S], BF16, kind="ExternalInput")
    wq = nc.dram_tensor("wqt", [D, MG], F32, kind="ExternalInput")  # w_q[rows_g].T
    wk = nc.dram_tensor("wkt", [D, MG], F32, kind="ExternalInput")
    wv = nc.dram_tensor("wvt", [D, MG], F32, kind="ExternalInput")
    wo = nc.dram_tensor("wot", [MG, D], F32, kind="ExternalInput")  # w_o[:, cols_g].T
    bq = nc.dram_tensor("bq", [MG], F32, kind="ExternalInput")
    bk = nc.dram_tensor("bk", [MG], F32, kind="ExternalInput")
    bv = nc.dram_tensor("bv", [MG], F32, kind="ExternalInput")
    yt = nc.dram_tensor("yt", [D, S], BF16, kind="ExternalOutput")  # partial y[b].T
    dbg = None
    if os.environ.get("KERNEL_DEBUG_TAPS"):
        dbg = {
            "dbg_kt": nc.dram_tensor("dbg_kt", [128, 512], BF16, kind="ExternalOutput"),
            "dbg_vt": nc.dram_tensor("dbg_vt", [128, 130], BF16, kind="ExternalOutput"),
            "dbg_pt": nc.dram_tensor("dbg_pt", [128, 1024], BF16, kind="ExternalOutput"),
            "dbg_ot": nc.dram_tensor("dbg_ot", [128, 512], BF16, kind="ExternalOutput"),
        }

    SC = 512  # s-chunk width (query positions per attention unit)
    n_sc = S // SC
    n_tc = S // 128  # key blocks of 128
    VTG = min(1024, S)  # v-projection t group width
    n_vtg = S // VTG
    ND = D // 128  # d-model k-tiles

    with tile.TileContext(nc) as tc, ExitStack() as ctx:
        persist = ctx.enter_context(tc.tile_pool(name="persist", bufs=1))

        # -------- weights / biases to SBUF --------
        wq_s = persist.tile([128, ND, MG], BF16, tag="wq")
        wk_s = persist.tile([128, ND, MG], BF16, tag="wk")
        wv_s = persist.tile([128, ND, MG], BF16, tag="wv")
        wo_s = persist.tile([128, MG // 128, D], BF16, tag="wo")
        bq_c = persist.tile([128, 2], F32, tag="bqc")  # per-partition cols
        bk_c = persist.tile([128, 2], F32, tag="bkc")
        bv_r = persist.tile([1, MG], F32, tag="bvr")
        bvb = persist.tile([128, MG], F32, tag="bvb")  # bv bcast over parts
        nc.gpsimd.dma_start(wq_s[:], wq[:].rearrange("(d p) m -> p d m", p=128))
        nc.gpsimd.dma_start(wk_s[:], wk[:].rearrange("(d p) m -> p d m", p=128))
        nc.gpsimd.dma_start(wv_s[:], wv[:].rearrange("(d p) m -> p d m", p=128))
        nc.gpsimd.dma_start(wo_s[:], wo[:].rearrange("(t p) n -> p t n", p=128))
        nc.gpsimd.dma_start(bq_c[:], bq[:].rearrange("(c p) -> p c", p=128))
        nc.gpsimd.dma_start(bk_c[:], bk[:].rearrange("(c p) -> p c", p=128))
        nc.gpsimd.dma_start(bv_r[:], bv[:].unsqueeze(0))
        nc.gpsimd.partition_broadcast(bvb[:], bv_r[:])

        # -------- persistent activations --------
        # Q^T/K^T: per (head pair hp, s-chunk) tiles [128, SC]
        qts = [
            [persist.tile([128, SC], BF16, tag=f"qt{i}_{j}", name=f"qt{i}_{j}") for j in range(n_sc)]
            for i in range(2)
        ]
        kts = [
            [persist.tile([128, SC], BF16, tag=f"kt{i}_{j}", name=f"kt{i}_{j}") for j in range(n_sc)]
            for i in range(2)
        ]
        # V: per (hp, t-block) tiles [t=128, 2*(64+1)]; col 64 of each
        # head's 65-col group = ones (softmax denominator row)
        vst = [
            [persist.tile([128, 2 * (DH + 1)], BF16, tag=f"vs{i}_{j}", name=f"vs{i}_{j}")
             for j in range(n_tc)]
            for i in range(2)
        ]
        # O^T (normalized): per (hp, s-chunk) tiles [128, SC]
        ott = [
            [persist.tile([128, SC], BF16, tag=f"ot{i}_{j}", name=f"ot{i}_{j}") for j in range(n_sc)]
            for i in range(2)
        ]

        for i in range(2):
            for j in range(n_tc):
                nc.vector.memset(vst[i][j][:, DH : DH + 1], 1.0)
                nc.vector.memset(vst[i][j][:, 2 * DH + 1 : 2 * DH + 2], 1.0)

        with tc.tile_pool(name="v_stage", bufs=9) as vstage, tc.tile_pool(
            name="x_stage", bufs=20
        ) as stage, tc.tile_pool(
            name="qk_psum", bufs=2, space="PSUM"
        ) as qkp, tc.tile_pool(
            name="pv_psum", bufs=4, space="PSUM"
        ) as pvp, tc.tile_pool(name="pt_pool", bufs=4) as ptp, tc.tile_pool(
            name="norm", bufs=4
        ) as normp, tc.tile_pool(name="y_stage", bufs=4) as ysp:

            # -------- phase B: K projection (sync DMA queue), then Q chunk 0 --------
            def proj_chunk(xin, w_s, b_c, dest, si, dmaq, x_pre=None):
                x_t = x_pre if x_pre is not None else load_x(xin, si, dmaq)
                ps = qkp.tile([128, 2 * SC], F32, tag="qk", name="ps")
                for mc in range(2):
                    for d in range(ND):
                        nc.tensor.matmul(
                            ps[:, mc * SC : (mc + 1) * SC],
                            w_s[:, d, mc * 128 : (mc + 1) * 128],
                            x_t[d][:],
                            start=(d == 0),
                            stop=(d == ND - 1),
                        )
                for mc in range(2):
                    nc.vector.tensor_scalar_add(
                        dest[mc][si][:],
                        ps[:, mc * SC : (mc + 1) * SC],
                        b_c[:, mc : mc + 1],
                    )

            def load_x(xin, si, dmaq):
                x_t = []
                for d in range(ND):
                    t1 = stage.tile([128, SC], BF16, tag="xs", name="xs")
                    dmaq.dma_start(
                        t1[:], xin[d * 128 : (d + 1) * 128, si * SC : (si + 1) * SC]
                    )
                    x_t.append(t1)
                return x_t

            for si in range(n_sc):
                proj_chunk(xk, wk_s, bk_c, kts, si, nc.sync)
            proj_chunk(xq, wq_s, bq_c, qts, 0, nc.sync)

            # -------- phase A: V projection (natural [t, m] layout) --------
            for tg in range(n_vtg):
                xv_t = []
                for d in range(ND):
                    t1 = vstage.tile([128, VTG], BF16, tag="xv", name="xv")
                    nc.gpsimd.dma_start(
                        t1[:], xv[d * 128 : (d + 1) * 128, tg * VTG : (tg + 1) * VTG]
                    )
                    xv_t.append(t1)
                for tl in range(VTG // 128):
                    ps = qkp.tile([128, 2 * SC], F32, tag="qk")
                    for d in range(ND):
                        nc.tensor.matmul(
                            ps[:, 0:MG],
                            xv_t[d][:, tl * 128 : (tl + 1) * 128],
                            wv_s[:, d, :],
                            start=(d == 0),
                            stop=(d == ND - 1),
                        )
                    tcix = tg * (VTG // 128) + tl
                    for hp in range(2):
                        for hh in range(2):
                            mlo = hp * 128 + hh * DH
                            nc.vector.tensor_tensor(
                                vst[hp][tcix][:, hh * (DH + 1) : hh * (DH + 1) + DH],
                                ps[:, mlo : mlo + DH],
                                bvb[:, mlo : mlo + DH],
                                ALU.add,
                            )

            if dbg is not None:
                nc.sync.dma_start(dbg["dbg_kt"][:], kts[0][0][:])
                nc.sync.dma_start(dbg["dbg_vt"][:], vst[0][0][:])

            # -------- phase C: attention (sc outer, hp inner) + out-proj --------
            xq_pre = None
            for sc in range(n_sc):
                # prefetch next chunk's xq tiles before this unit's attention
                xq_next = load_x(xq, sc + 1, nc.sync) if sc + 1 < n_sc else None
                for hp in range(2):
                    pv = [pvp.tile([DH + 1, SC], F32, tag="pv", name="pv") for _ in range(2)]
                    prev = None  # (tcix, pt) lag so Act never blocks PE FIFO
                    for tcix in range(n_tc):
                        kt_tile = kts[hp][(tcix * 128) // SC]
                        kcol = (tcix * 128) % SC
                        qt_tile = qts[hp][sc]
                        qk = qkp.tile([128, 2 * SC], F32, tag="qk")
                        # both heads' K=64 scores back-to-back -> row-tiled
                        # concurrent on PE (base partitions 0 / 64)
                        nc.tensor.matmul(
                            qk[:, 0:SC],
                            kt_tile[0:DH, kcol : kcol + 128],
                            qt_tile[0:DH, :],
                            start=True,
                            stop=True,
                        )
                        nc.tensor.matmul(
                            qk[:, SC : 2 * SC],
                            kt_tile[DH:128, kcol : kcol + 128],
                            qt_tile[DH:128, :],
                            start=True,
                            stop=True,
                        )
                        pt = ptp.tile([128, 2 * SC], BF16, tag="pt")
                        nc.scalar.activation(pt[:], qk[:], AF.Exp, scale=0.125)
                        if dbg is not None and sc == 0 and hp == 0 and tcix == 0:
                            nc.sync.dma_start(dbg["dbg_pt"][:], pt[:])
                        if prev is not None:
                            ptc, ppt = prev
                            for hh in range(2):
                                nc.tensor.matmul(
                                    pv[hh][:],
                                    vst[hp][ptc][:, hh * (DH + 1) : (hh + 1) * (DH + 1)],
                                    ppt[:, hh * SC : (hh + 1) * SC],
                                    start=(ptc == 0),
                                    stop=(ptc == n_tc - 1),
                                )
                        prev = (tcix, pt)
                    ptc, ppt = prev
                    for hh in range(2):
                        nc.tensor.matmul(
                            pv[hh][:],
                            vst[hp][ptc][:, hh * (DH + 1) : (hh + 1) * (DH + 1)],
                            ppt[:, hh * SC : (hh + 1) * SC],
                            start=(ptc == 0),
                            stop=(ptc == n_tc - 1),
                        )
                    for hh in range(2):
                        dsb = normp.tile([1, SC], F32, tag="dsb", name="dsb")
                        nc.vector.tensor_copy(dsb[:], pv[hh][DH : DH + 1, :])
                        rd = normp.tile([1, SC], F32, tag="rd")
                        nc.vector.reciprocal_approx_fast(rd[:], dsb[:])
                        rdb = normp.tile([DH, SC], F32, tag="rdb")
                        nc.gpsimd.partition_broadcast(rdb[:], rd[:])
                        nc.vector.tensor_tensor(
                            ott[hp][sc][hh * DH : (hh + 1) * DH, :],
                            pv[hh][0:DH, :],
                            rdb[:],
                            ALU.mult,
                        )
                if dbg is not None and sc == 0:
                    nc.sync.dma_start(dbg["dbg_ot"][:], ott[0][0][:])
                # Q projection for the next chunk rides in PE slack
                if sc + 1 < n_sc:
                    proj_chunk(xq, wq_s, bq_c, qts, sc + 1, nc.sync, x_pre=xq_next)
                # out-projection for this sc (borrows qk psum slots)
                for dp in range(ND // 2):
                    ps = qkp.tile([128, 2 * SC], F32, tag="qk")
                    for half in range(2):
                        d8 = dp * 2 + half
                        for mt in range(MG // 128):
                            nc.tensor.matmul(
                                ps[:, half * SC : (half + 1) * SC],
                                wo_s[:, mt, d8 * 128 : (d8 + 1) * 128],
                                ott[mt][sc][:],
                                start=(mt == 0),
                                stop=(mt == MG // 128 - 1),
                            )
                    ys = ysp.tile([128, 2 * SC], BF16, tag="ys")
                    nc.vector.tensor_copy(ys[:], ps[:])
                    for half in range(2):
                        d8 = dp * 2 + half
                        nc.gpsimd.dma_start(
                            yt[d8 * 128 : (d8 + 1) * 128, sc * SC : (sc + 1) * SC],
                            ys[:, half * SC : (half + 1) * SC],
                        )

    nc.compile()
    return nc


_MODULE_CACHE: dict = {}


def _get_module(S: int) -> bass.Bass:
    if S not in _MODULE_CACHE:
        _MODULE_CACHE[S] = build_module(S)
    return _MODULE_CACHE[S]


def make_in_maps(q, k, v, w_q, b_q, w_k, b_k, w_v, b_v, w_o, b_o):
    """Shard full inputs into 8 per-core input maps (host-side prep)."""
    f = lambda a: np.ascontiguousarray(np.asarray(a, dtype=np.float32))
    w_q, w_k, w_v, w_o = f(w_q), f(w_k), f(w_v), f(w_o)
    b_q, b_k, b_v = f(b_q), f(b_k), f(b_v)
    # per-batch transposed bf16 activations, shared across the 4 cores of b
    xqt = [np.ascontiguousarray(np.asarray(q)[b].T.astype(BF16NP)) for b in range(2)]
    xkt = [np.ascontiguousarray(np.asarray(k)[b].T.astype(BF16NP)) for b in range(2)]
    xvt = [np.ascontiguousarray(np.asarray(v)[b].T.astype(BF16NP)) for b in range(2)]
    in_maps = []
    for core in range(8):
        b, g = core // 4, core % 4
        rows = slice(g * MG, (g + 1) * MG)
        in_maps.append(
            {
                "xqt": xqt[b],
                "xkt": xkt[b],
                "xvt": xvt[b],
                "wqt": np.ascontiguousarray(w_q[rows].T),
                "wkt": np.ascontiguousarray(w_k[rows].T),
                "wvt": np.ascontiguousarray(w_v[rows].T),
                "wot": np.ascontiguousarray(w_o[:, rows].T),
                "bq": np.ascontiguousarray(b_q[rows]),
                "bk": np.ascontiguousarray(b_k[rows]),
                "bv": np.ascontiguousarray(b_v[rows]),
            }
        )
    return in_maps


def gather_output(results, b_o, B, S):
    y = np.zeros((B, S, D), np.float32)
    for core in range(8):
        b = core // 4
        y[b] += results[core]["yt"].astype(np.float32).T
    y += np.asarray(b_o, np.float32)[None, None, :]
    return y


def run(inputs: dict, trace: bool = False):
    """Run on 8 NeuronCores; returns (y, BassKernelResults)."""
    from concourse import bass_utils

    B, S, _ = np.asarray(inputs["q"]).shape
    mod = _get_module(S)
    in_maps = make_in_maps(**inputs)
    res = bass_utils.run_bass_kernel_spmd(
        mod, in_maps, core_ids=list(range(8)), trace=trace
    )
    y = gather_output(res.results, inputs["b_o"], B, S)
    return y, res


def kernel(q, k, v, w_q, b_q, w_k, b_k, w_v, b_v, w_o, b_o):
    y, _ = run(
        dict(
            q=q, k=k, v=v, w_q=w_q, b_q=b_q, w_k=w_k, b_k=b_k,
            w_v=w_v, b_v=b_v, w_o=w_o, b_o=b_o,
        )
    )
    return y


# revision 13
# speedup vs baseline: 1.3614x; 1.0037x over previous
"""Trainium2 Bass kernel for 16-head MHA (B=2, S=4096, D=1024).

Sharding: 8 cores = 2 batches x 4 head-groups (4 heads each).
Each core computes, for its (batch b, head group g):
    Q^T/K^T ([256, S] head-major), V ([S, 4*(64+1)] with ones cols),
    per head pair: S^T = K Q^T for both heads via concurrent row-tiled
    (K=64) matmuls into one [128, 1024] PSUM tile, one fused Exp over
    both heads, [O^T; D] = [V|1]^T P^T per head, O^T/D, Y^T = woT^T O^T.
Host sums the 4 per-head-group partials per batch and adds b_o.

Perf notes (from NTFF profile analysis):
  - PE matmul cost = N streamed rows regardless of K/M; the two K=64
    score matmuls of a head pair run CONCURRENTLY when issued
    back-to-back with lhsT/rhs at base partitions 0 and 64 (row
    tiling).  This halves score cost vs. serial emission.
  - The Act engine (only engine with Exp) is the bottleneck:
    512 x [128,1024] Exp instructions ~= 570us.  Keep everything else
    (copies, biases) off it: biases are folded into DVE evacuation.
  - PV is emitted lagged one t-block so the PE FIFO never blocks
    behind an Act-dependent instruction; Act runs back-to-back.
  - x/y are bf16 host-side to halve HBM traffic.
"""

import os
import sys

sys.path.insert(0, "/opt/trn_rl_repo")
os.environ.setdefault("MYCRO_LOCAL_CACHE", "1")

from contextlib import ExitStack

import ml_dtypes
import numpy as np

import concourse.bass as bass
import concourse.tile as tile
from concourse import bacc, mybir

F32 = mybir.dt.float32
BF16 = mybir.dt.bfloat16
AF = mybir.ActivationFunctionType
ALU = mybir.AluOpType

D = 1024  # d_model
NH = 16  # total heads
DH = 64  # head dim
HPC = 4  # heads per core
MG = HPC * DH  # 256 model cols per core

BF16NP = ml_dtypes.bfloat16


def build_module(S: int = 4096) -> bass.Bass:
    nc = bacc.Bacc("TRN2", target_bir_lowering=False, debug=False, num_devices=8)

    xq = nc.dram_tensor("xqt", [D, S], BF16, kind="ExternalInput")  # q[b].T
    xk = nc.dram_tensor("xkt", [D, S], BF16, kind="ExternalInput")
    xv = nc.dram_tensor("xvt", [D, S], BF16, kind="ExternalInput")
    wq = nc.dram_tensor("wqt", [D, MG], F32, kind="ExternalInput")  # w_q[rows_g].T
    wk = nc.dram_tensor("wkt", [D, MG], F32, kind="ExternalInput")
    wv = nc.dram_tensor("wvt", [D, MG], F32, kind="ExternalInput")
    wo = nc.dram_tensor("wot", [MG, D], F32, kind="ExternalInput")  # w_o[:, cols_g].T
    bq = nc.dram_tensor("bq", [MG], F32, kind="ExternalInput")
    bk = nc.dram_tensor("bk", [MG], F32, kind="ExternalInput")
    bv = nc.dram_tensor("bv", [MG], F32, kind="ExternalInput")
    yt = nc.dram_tensor("yt", [D, S], BF16, kind="ExternalOutput")  # partial y[b].T
    dbg = None
    if os.environ.get("KERNEL_DEBUG_TAPS"):
        dbg = {
            "dbg_kt": nc.dram_tensor("dbg_kt", [128, 512], BF16, kind="ExternalOutput"),
            "dbg_vt": nc.dram_tensor("dbg_vt", [128, 130], BF16, kind="ExternalOutput"),
            "dbg_pt": nc.dram_tensor("dbg_pt", [128, 1024], BF16, kind="ExternalOutput"),
            "dbg_ot": nc.dram_tensor("dbg_ot", [128, 512], BF16, kind="ExternalOutput"),
        }

    SC = 512  # s-chunk width (query positions per attention unit)
    n_sc = S // SC
    n_tc = S // 128  # key blocks of 128
    VTG = min(1024, S)  # v-projection t group width
    n_vtg = S // VTG
    ND = D // 128  # d-model k-tiles

    with tile.TileContext(nc) as tc, ExitStack() as ctx:
        persist = ctx.enter_context(tc.tile_pool(name="persist", bufs=1))

        # -------- weights / biases to SBUF --------
        wq_s = persist.tile([128, ND, MG], BF16, tag="wq")
        wk_s = persist.tile([128, ND, MG], BF16, tag="wk")
        wv_s = persist.tile([128, ND, MG], BF16, tag="wv")
        wo_s = persist.tile([128, MG // 128, D], BF16, tag="wo")
        bq_c = persist.tile([128, 2], F32, tag="bqc")  # per-partition cols
        bk_c = persist.tile([128, 2], F32, tag="bkc")
        bv_r = persist.tile([1, MG], F32, tag="bvr")
        bvb = persist.tile([128, MG], F32, tag="bvb")  # bv bcast over parts
        nc.gpsimd.dma_start(wq_s[:], wq[:].rearrange("(d p) m -> p d m", p=128))
        nc.gpsimd.dma_start(wk_s[:], wk[:].rearrange("(d p) m -> p d m", p=128))
        nc.gpsimd.dma_start(wv_s[:], wv[:].rearrange("(d p) m -> p d m", p=128))
        nc.gpsimd.dma_start(wo_s[:], wo[:].rearrange("(t p) n -> p t n", p=128))
        nc.gpsimd.dma_start(bq_c[:], bq[:].rearrange("(c p) -> p c", p=128))
        nc.gpsimd.dma_start(bk_c[:], bk[:].rearrange("(c p) -> p c", p=128))
        nc.gpsimd.dma_start(bv_r[:], bv[:].unsqueeze(0))
        nc.gpsimd.partition_broadcast(bvb[:], bv_r[:])

        # -------- persistent activations --------
        # Q^T/K^T: per (head pair hp, s-chunk) tiles [128, SC]
        qts = [
            [persist.tile([128, SC], BF16, tag=f"qt{i}_{j}", name=f"qt{i}_{j}") for j in range(n_sc)]
            for i in range(2)
        ]
        kts = [
            [persist.tile([128, SC], BF16, tag=f"kt{i}_{j}", name=f"kt{i}_{j}") for j in range(n_sc)]
            for i in range(2)
        ]
        # V: per (hp, t-block) tiles [t=128, 2*(64+1)]; col 64 of each
        # head's 65-col group = ones (softmax denominator row)
        vst = [
            [persist.tile([128, 2 * (DH + 1)], BF16, tag=f"vs{i}_{j}", name=f"vs{i}_{j}")
             for j in range(n_tc)]
            for i in range(2)
        ]
        # O^T (normalized): per (hp, s-chunk) tiles [128, SC]
        ott = [
            [persist.tile([128, SC], BF16, tag=f"ot{i}_{j}", name=f"ot{i}_{j}") for j in range(n_sc)]
            for i in range(2)
        ]

        for i in range(2):
            for j in range(n_tc):
                nc.vector.memset(vst[i][j][:, DH : DH + 1], 1.0)
                nc.vector.memset(vst[i][j][:, 2 * DH + 1 : 2 * DH + 2], 1.0)

        with tc.tile_pool(name="v_stage", bufs=9) as vstage, tc.tile_pool(
            name="x_stage", bufs=20
        ) as stage, tc.tile_pool(
            name="qk_psum", bufs=2, space="PSUM"
        ) as qkp, tc.tile_pool(
            name="pv_psum", bufs=4, space="PSUM"
        ) as pvp, tc.tile_pool(name="pt_pool", bufs=4) as ptp, tc.tile_pool(
            name="norm", bufs=4
        ) as normp, tc.tile_pool(name="y_stage", bufs=4) as ysp:

            # -------- phase B: K projection (sync DMA queue), then Q chunk 0 --------
            def proj_chunk(xin, w_s, b_c, dest, si, dmaq, x_pre=None):
                x_t = x_pre if x_pre is not None else load_x(xin, si, dmaq)
                ps = qkp.tile([128, 2 * SC], F32, tag="qk", name="ps")
                for mc in range(2):
                    for d in range(ND):
                        nc.tensor.matmul(
                            ps[:, mc * SC : (mc + 1) * SC],
                            w_s[:, d, mc * 128 : (mc + 1) * 128],
                            x_t[d][:],
                            start=(d == 0),
                            stop=(d == ND - 1),
                        )
                for mc in range(2):
                    nc.vector.tensor_scalar_add(
                        dest[mc][si][:],
                        ps[:, mc * SC : (mc + 1) * SC],
                        b_c[:, mc : mc + 1],
                    )

            def load_x(xin, si, dmaq):
                x_t = []
                for d in range(ND):
                    t1 = stage.tile([128, SC], BF16, tag="xs", name="xs")
                    dmaq.dma_start(
                        t1[:], xin[d * 128 : (d + 1) * 128, si * SC : (si + 1) * SC]
                    )
                    x_t.append(t1)
                return x_t

            for si in range(n_sc):
                proj_chunk(xk, wk_s, bk_c, kts, si, nc.sync)
            proj_chunk(xq, wq_s, bq_c, qts, 0, nc.sync)

            # -------- phase A: V projection (natural [t, m] layout) --------
            for tg in range(n_vtg):
                xv_t = []
                for d in range(ND):
                    t1 = vstage.tile([128, VTG], BF16, tag="xv", name="xv")
                    nc.gpsimd.dma_start(
                        t1[:], xv[d * 128 : (d + 1) * 128, tg * VTG : (tg + 1) * VTG]
                    )
                    xv_t.append(t1)
                for tl in range(VTG // 128):
                    ps = qkp.tile([128, 2 * SC], F32, tag="qk")
                    for d in range(ND):
                        nc.tensor.matmul(
                            ps[:, 0:MG],
                            xv_t[d][:, tl * 128 : (tl + 1) * 128],
                            wv_s[:, d, :],
                            start=(d == 0),
                            stop=(d == ND - 1),
                        )
                    tcix = tg * (VTG // 128) + tl
                    for hp in range(2):
                        for hh in range(2):
                            mlo = hp * 128 + hh * DH
                            nc.vector.tensor_tensor(
                                vst[hp][tcix][:, hh * (DH + 1) : hh * (DH + 1) + DH],
                                ps[:, mlo : mlo + DH],
                                bvb[:, mlo : mlo + DH],
                                ALU.add,
                            )

            if dbg is not None:
                nc.sync.dma_start(dbg["dbg_kt"][:], kts[0][0][:])
                nc.sync.dma_start(dbg["dbg_vt"][:], vst[0][0][:])

            # -------- phase C: attention (sc outer, hp inner) + out-proj --------
            xq_pre = None
            for sc in range(n_sc):
                # prefetch next chunk's xq tiles before this unit's attention
                xq_next = load_x(xq, sc + 1, nc.sync) if sc + 1 < n_sc else None
                for hp in range(2):
                    pv = [pvp.tile([DH + 1, SC], F32, tag="pv", name="pv") for _ in range(2)]
                    prev = None  # (tcix, pt) lag so Act never blocks PE FIFO
                    for tcix in range(n_tc):
                        kt_tile = kts[hp][(tcix * 128) // SC]
                        kcol = (tcix * 128) % SC
                        qt_tile = qts[hp][sc]
                        qk = qkp.tile([128, 2 * SC], F32, tag="qk")
                        # both heads' K=64 scores back-to-back -> row-tiled
                        # concurrent on PE (base partitions 0 / 64)
                        nc.tensor.matmul(
                            qk[:, 0:SC],
                            kt_tile[0:DH, kcol : kcol + 128],
                            qt_tile[0:DH, :],
                            start=True,
                            stop=True,
                        )
                        nc.tensor.matmul(
                            qk[:, SC : 2 * SC],
                            kt_tile[DH:128, kcol : kcol + 128],
                            qt_tile[DH:128, :],
                            start=True,
                            stop=True,
                        )
                        pt = ptp.tile([128, 2 * SC], BF16, tag="pt")
                        nc.scalar.activation(pt[:], qk[:], AF.Exp, scale=0.125)
                        if dbg is not None and sc == 0 and hp == 0 and tcix == 0:
                            nc.sync.dma_start(dbg["dbg_pt"][:], pt[:])
                        if prev is not None:
                            ptc, ppt = prev
                            for hh in range(2):
                                nc.tensor.matmul(
                                    pv[hh][:],
                                    vst[hp][ptc][:, hh * (DH + 1) : (hh + 1) * (DH + 1)],
                                    ppt[:, hh * SC : (hh + 1) * SC],
                                    start=(ptc == 0),
                                    stop=(ptc == n_tc - 1),
                                )
                        prev = (tcix, pt)
                    ptc, ppt = prev
                    for hh in range(2):
                        nc.tensor.matmul(
                            pv[hh][:],
                            vst[hp][ptc][:, hh * (DH + 1) : (hh + 1) * (DH + 1)],
                            ppt[:, hh * SC : (hh + 1) * SC],
                            start=(ptc == 0),
                            stop=(ptc == n_tc - 1),
                        )
                    for hh in range(2):
                        dsb = normp.tile([1, SC], F32, tag="dsb", name="dsb")
                        nc.vector.tensor_copy(dsb[:], pv[hh][DH : DH + 1, :])
                        rd = normp.tile([1, SC], F32, tag="rd")
                        nc.vector.reciprocal_approx_fast(rd[:], dsb[:])
                        rdb = normp.tile([DH, SC], F32, tag="rdb")
                        nc.gpsimd.partition_broadcast(rdb[:], rd[:])
                        nc.vector.tensor_tensor(
                            ott[hp][sc][hh * DH : (hh + 1) * DH, :],
                            pv[hh][0:DH, :],
                            rdb[:],
                            ALU.mult,
                        )
                if dbg is not None and sc == 0:
                    nc.sync.dma_start(dbg["dbg_ot"][:], ott[0][0][:])
                # Q projection for the next chunk rides in PE slack
                if sc + 1 < n_sc:
                    proj_chunk(xq, wq_s, bq_c, qts, sc + 1, nc.sync, x_pre=xq_next)
                # out-projection for this sc (borrows qk psum slots)
                for dp in range(ND // 2):
                    ps = qkp.tile([128, 2 * SC], F32, tag="qk")
                    for half in range(2):
                        d8 = dp * 2 + half
                        for mt in range(MG // 128):
                            nc.tensor.matmul(
                                ps[:, half * SC : (half + 1) * SC],
                                wo_s[:, mt, d8 * 128 : (d8 + 1) * 128],
                                ott[mt][sc][:],
                                start=(mt == 0),
                                stop=(mt == MG // 128 - 1),
                            )
                    ys = ysp.tile([128, 2 * SC], BF16, tag="ys")
                    nc.vector.tensor_copy(ys[:], ps[:])
                    for half in range(2):
                        d8 = dp * 2 + half
                        nc.gpsimd.dma_start(
                            yt[d8 * 128 : (d8 + 1) * 128, sc * SC : (sc + 1) * SC],
                            ys[:, half * SC : (half + 1) * SC],
                        )

    nc.compile()
    return nc


_MODULE_CACHE: dict = {}


def _get_module(S: int) -> bass.Bass:
    if S not in _MODULE_CACHE:
        _MODULE_CACHE[S] = build_module(S)
    return _MODULE_CACHE[S]


def make_in_maps(q, k, v, w_q, b_q, w_k, b_k, w_v, b_v, w_o, b_o):
    """Shard full inputs into 8 per-core input maps (host-side prep)."""
    f = lambda a: np.ascontiguousarray(np.asarray(a, dtype=np.float32))
    w_q, w_k, w_v, w_o = f(w_q), f(w_k), f(w_v), f(w_o)
    b_q, b_k, b_v = f(b_q), f(b_k), f(b_v)
    # per-batch transposed bf16 activations, shared across the 4 cores of b
    xqt = [np.ascontiguousarray(np.asarray(q)[b].T.astype(BF16NP)) for b in range(2)]
    xkt = [np.ascontiguousarray(np.asarray(k)[b].T.astype(BF16NP)) for b in range(2)]
    xvt = [np.ascontiguousarray(np.asarray(v)[b].T.astype(BF16NP)) for b in range(2)]
    in_maps = []
    for core in range(8):
        b, g = core // 4, core % 4
        rows = slice(g * MG, (g + 1) * MG)
        in_maps.append(
            {
                "xqt": xqt[b],
                "xkt": xkt[b],
                "xvt": xvt[b],
                "wqt": np.ascontiguousarray(w_q[rows].T),
                "wkt": np.ascontiguousarray(w_k[rows].T),
                "wvt": np.ascontiguousarray(w_v[rows].T),
                "wot": np.ascontiguousarray(w_o[:, rows].T),
                "bq": np.ascontiguousarray(b_q[rows]),
                "bk": np.ascontiguousarray(b_k[rows]),
                "bv": np.ascontiguousarray(b_v[rows]),
            }
        )
    return in_maps


def gather_output(results, b_o, B, S):
    y = np.zeros((B, S, D), np.float32)
    for core in range(8):
        b = core // 4
        y[b] += results[core]["yt"].astype(np.float32).T
    y += np.asarray(b_o, np.float32)[None, None, :]
    return y


def run(inputs: dict, trace: bool = False):
    """Run on 8 NeuronCores; returns (y, BassKernelResults)."""
    from concourse import bass_utils

    B, S, _ = np.asarray(inputs["q"]).shape
    mod = _get_module(S)
    in_maps = make_in_maps(**inputs)
    res = bass_utils.run_bass_kernel_spmd(
        mod, in_maps, core_ids=list(range(8)), trace=trace
    )
    y = gather_output(res.results, inputs["b_o"], B, S)
    return y, res


def kernel(q, k, v, w_q, b_q, w_k, b_k, w_v, b_v, w_o, b_o):
    y, _ = run(
        dict(
            q=q, k=k, v=v, w_q=w_q, b_q=b_q, w_k=w_k, b_k=b_k,
            w_v=w_v, b_v=b_v, w_o=w_o, b_o=b_o,
        )
    )
    return y
